# revision 2
# baseline (speedup 1.0000x reference)
"""Trainium2 Bass kernel for causal self-attention with RoPE (fused pipeline).

Sharding: 8 cores = 2 batches x 4 head-groups (4 heads each).
Each core: qkv projection (bf16) for its heads, RoPE, causal flash
attention, partial output projection; host sums the 4 bf16 partials per
batch in f32.

Single fused emission stream keeps the PE continuously busy (pstate ramp):
qkv(0), qkv(1), attn(0), qkv(2), attn(1)+proj(0), qkv(3), attn(2)+proj(1),
attn(3)+proj(2), proj(3).

exp is split between the ACT engine (native Exp) and the DVE (Schraudolph
bit-trick via tensor_scalar f32->uint16, viewed as bf16); the causal mask
rides in the Schraudolph bias operand so no PE mask matmuls are needed.
"""

import os

import numpy as np

NUM_HEADS = 16
B, T, C = 2, 2048, 1024
D = C // NUM_HEADS  # 64
HPC = 4             # heads per core
NCORES = 8

# Schraudolph exp constants for bf16 output (validated on HW: max rel ~3.9%)
A16 = float(2 ** 7 / np.log(2))
B16 = 16249.13
MBIG = 12000.0  # mask subtraction folded into the bias operand

_CACHE = {}

LAST_EXEC_NS = None


def _build_body(nc, reps=1):
    import concourse.bass as bass
    import concourse.mybir as mybir
    import concourse.tile as tile
    from contextlib import ExitStack

    F32 = mybir.dt.float32
    BF16 = mybir.dt.bfloat16
    U16 = mybir.dt.uint16
    AF = mybir.ActivationFunctionType
    ALU = mybir.AluOpType

    xT = nc.dram_tensor("xT", [C, T], BF16, kind="ExternalInput").ap()
    wT = nc.dram_tensor("wT", [C, 768], BF16, kind="ExternalInput").ap()
    projT = nc.dram_tensor("projT", [256, C], BF16, kind="ExternalInput").ap()
    CS = nc.dram_tensor("CS", [128, T], F32, kind="ExternalInput").ap()
    SN = nc.dram_tensor("SN", [128, T], F32, kind="ExternalInput").ap()
    BM = nc.dram_tensor("BM", [128, 512], F32, kind="ExternalInput").ap()
    ident = nc.dram_tensor("ident", [128, 128], F32, kind="ExternalInput").ap()
    out = nc.dram_tensor("out", [T, C], BF16, kind="ExternalOutput").ap()
    linv_dram = nc.dram_tensor("linv_scratch", [64, 128], BF16).ap()

    with tile.TileContext(nc) as tc, ExitStack() as ctx:
        singles = ctx.enter_context(tc.tile_pool(name="singles", bufs=1))
        stream = ctx.enter_context(tc.tile_pool(name="stream", bufs=2))
        ps = ctx.enter_context(tc.tile_pool(name="ps", bufs=1, space="PSUM"))

        # ---- persistent SBUF ----
        w_sb = singles.tile([128, 8, 768], BF16)
        cs_sb = singles.tile([128, T], F32)
        sn_sb = singles.tile([128, T], F32)
        bm_sb = singles.tile([128, 512], F32)
        id_sb = singles.tile([128, 128], F32)
        pj_sb = singles.tile([128, 2, C], BF16)
        q_rot = singles.tile([128, 2, T], BF16)
        k_rot = singles.tile([128, 2, T], BF16)
        v_sb = singles.tile([128, 16, 65 * HPC], BF16)
        u_sb = singles.tile([65, 16, 512], F32)
        u2_sb = singles.tile([128, 2, T], BF16)
        l_sb = singles.tile([128, 512], F32)
        linv_col = singles.tile([128, 64], F32)

        # weight slices first: ci0 on gpsimd, ci1 on scalar so the first
        # matmul's stationary arrives in ~2us; the rest stream behind.
        for ci in range(8):
            eng = nc.gpsimd if ci % 2 == 0 else nc.scalar
            eng.dma_start(out=w_sb[:, ci, :], in_=wT[ci * 128:(ci + 1) * 128, :])
        nc.gpsimd.dma_start(out=cs_sb[:], in_=CS)
        nc.gpsimd.dma_start(out=sn_sb[:], in_=SN)
        nc.scalar.dma_start(out=id_sb[:], in_=ident)
        nc.scalar.dma_start(out=bm_sb[:], in_=BM)
        for hpi in range(2):
            nc.scalar.dma_start(
                out=pj_sb[:, hpi, :], in_=projT[hpi * 128:(hpi + 1) * 128, :]
            )

        v_h = v_sb[:].rearrange("p t (h c) -> p t h c", c=65)
        nc.vector.memset(v_h[:, :, :, 64:65], 1.0)

        # round-robin engine pickers
        flex_state = {"i": 0}

        def flex_exp():
            # 2 ACT : 1 DVE for the flexible exp blocks
            i = flex_state["i"]
            flex_state["i"] += 1
            return "act" if i % 3 != 2 else "dve"

        oq_state = {"i": 0}
        out_qs = [nc.sync, nc.gpsimd, nc.scalar]

        def out_queue():
            i = oq_state["i"]
            oq_state["i"] += 1
            return out_qs[i % 3]

        evac_state = {"i": 0}

        def evac_eng():
            i = evac_state["i"]
            evac_state["i"] += 1
            return i % 2 == 0  # True -> ACT

        for rep in range(reps):
            # ---------------- qkv chunk ----------------
            def qkv(ch):
                tok = slice(ch * 512, (ch + 1) * 512)
                xts = []
                for ci in range(8):
                    xt = stream.tile([128, 512], BF16, tag="xt", bufs=9,
                                     name="xt")
                    nc.sync.dma_start(
                        out=xt[:], in_=xT[ci * 128:(ci + 1) * 128, tok]
                    )
                    xts.append(xt)
                qk_ps = [
                    ps.tile([128, 512], F32, tag="gen", bufs=6, name="qkps")
                    for _ in range(4)
                ]  # q ft0, q ft1, k ft0, k ft1
                v_ps = [
                    ps.tile([128, 2, 256], F32, tag="gen", bufs=6, name="vps")
                    for _ in range(2)
                ]
                for ci in range(8):
                    xt = xts[ci]
                    for ft in range(2):
                        nc.tensor.matmul(
                            qk_ps[ft][:], w_sb[:, ci, ft * 128:(ft + 1) * 128],
                            xt[:], start=(ci == 0), stop=(ci == 7),
                        )
                        nc.tensor.matmul(
                            qk_ps[2 + ft][:],
                            w_sb[:, ci, 256 + ft * 128:256 + (ft + 1) * 128],
                            xt[:], start=(ci == 0), stop=(ci == 7),
                        )
                    for sub in range(2):
                        for ts in range(2):
                            nc.tensor.matmul(
                                v_ps[sub][:, ts, :],
                                xt[:, sub * 256 + ts * 128:sub * 256 + (ts + 1) * 128],
                                w_sb[:, ci, 512:768],
                                start=(ci == 0 and ts == 0),
                                stop=(ci == 7 and ts == 1),
                            )
                # ---- RoPE q on DVE (reads psum), swap on sync DMAs ----
                t1q = stream.tile([128, 2, 512], BF16, tag="t1q", name="t1q")
                t2q = stream.tile([128, 2, 512], BF16, tag="t2q", name="t2q")
                t2sq = stream.tile([128, 2, 512], BF16, tag="t2sq", name="t2sq")
                for ft in range(2):
                    nc.vector.tensor_mul(t1q[:, ft, :], qk_ps[ft][:], cs_sb[:, tok])
                    nc.vector.tensor_mul(t2q[:, ft, :], qk_ps[ft][:], sn_sb[:, tok])
                for hb in range(2):
                    e = slice(hb * 64, hb * 64 + 32)
                    o = slice(hb * 64 + 32, hb * 64 + 64)
                    nc.sync.dma_start(out=t2sq[e, :, :], in_=t2q[o, :, :])
                    nc.sync.dma_start(out=t2sq[o, :, :], in_=t2q[e, :, :])
                nc.vector.tensor_add(q_rot[:, :, tok], t1q[:], t2sq[:])
                # ---- RoPE k: ACT evacuates psum, Pool does the muls/add ----
                kc = stream.tile([128, 2, 512], BF16, tag="kc", name="kc")
                for ft in range(2):
                    nc.scalar.copy(kc[:, ft, :], qk_ps[2 + ft][:])
                t1k = stream.tile([128, 2, 512], BF16, tag="t1k", name="t1k")
                t2k = stream.tile([128, 2, 512], BF16, tag="t2k", name="t2k")
                t2sk = stream.tile([128, 2, 512], BF16, tag="t2sk", name="t2sk")
                for ft in range(2):
                    nc.gpsimd.tensor_mul(t1k[:, ft, :], kc[:, ft, :], cs_sb[:, tok])
                    nc.gpsimd.tensor_mul(t2k[:, ft, :], kc[:, ft, :], sn_sb[:, tok])
                for hb in range(2):
                    e = slice(hb * 64, hb * 64 + 32)
                    o = slice(hb * 64 + 32, hb * 64 + 64)
                    nc.gpsimd.dma_start(out=t2sk[e, :, :], in_=t2k[o, :, :])
                    nc.gpsimd.dma_start(out=t2sk[o, :, :], in_=t2k[e, :, :])
                nc.gpsimd.tensor_add(k_rot[:, :, tok], t1k[:], t2sk[:])
                # ---- v evacuation on ACT ----
                for sub in range(2):
                    for ts in range(2):
                        tokt = ch * 4 + sub * 2 + ts
                        nc.scalar.copy(
                            v_h[:, tokt, :, 0:64],
                            v_ps[sub][:, ts, :].rearrange(
                                "p (h c) -> p h c", h=4
                            ),
                        )

            # ---------------- attention ----------------
            def emit_scores(hp, qb, kt):
                j = kt - 4 * qb
                off = max(j, 0) * 128
                ks = slice(kt * 128, (kt + 1) * 128)
                qs = slice(qb * 512 + off, (qb + 1) * 512)
                sA = ps.tile([128, 512], F32, tag="gen", bufs=6, name="sA")
                sB = ps.tile([128, 512], F32, tag="gen", bufs=6, name="sB")
                nc.tensor.matmul(
                    sA[:, off:512], k_rot[0:64, hp, ks], q_rot[0:64, hp, qs],
                    start=True, stop=True,
                )
                nc.tensor.matmul(
                    sB[:, off:512], k_rot[64:128, hp, ks], q_rot[64:128, hp, qs],
                    start=True, stop=True,
                )
                return sA, sB, off, kt

            def emit_exp_pv(hp, qb, st, uA, uB, nkt):
                sA, sB, off, kt = st
                hA = 2 * hp
                hB = 2 * hp + 1
                diag = kt >= 4 * qb
                pAB = stream.tile([128, 1024], BF16, tag="pAB", bufs=3,
                                  name="pAB")
                for h, s_t in ((0, sA), (1, sB)):
                    p_sl = pAB[:, h * 512 + off:h * 512 + 512]
                    if diag:
                        n1 = min(128, 512 - off)
                        nc.vector.scalar_tensor_tensor(
                            pAB[:, h * 512 + off:h * 512 + off + n1].bitcast(U16),
                            s_t[:, off:off + n1], A16, bm_sb[:, 0:n1],
                            ALU.mult, ALU.add,
                        )
                        if off + n1 < 512:
                            tail = slice(off + n1, 512)
                            if flex_exp() == "act":
                                nc.scalar.activation(
                                    pAB[:, h * 512 + off + n1:h * 512 + 512],
                                    s_t[:, tail], AF.Exp,
                                )
                            else:
                                nc.vector.tensor_scalar(
                                    pAB[:, h * 512 + off + n1:h * 512 + 512]
                                    .bitcast(U16),
                                    s_t[:, tail], A16, B16, ALU.mult, ALU.add,
                                )
                    else:
                        if flex_exp() == "act":
                            nc.scalar.activation(p_sl, s_t[:, off:512], AF.Exp)
                        else:
                            nc.vector.tensor_scalar(
                                p_sl.bitcast(U16), s_t[:, off:512],
                                A16, B16, ALU.mult, ALU.add,
                            )
                nc.tensor.matmul(
                    uA[0:65, off:512], v_sb[:, kt, hA * 65:(hA + 1) * 65],
                    pAB[:, off:512], start=(kt == 0), stop=(kt == nkt - 1),
                )
                nc.tensor.matmul(
                    uB[0:65, off:512], v_sb[:, kt, hB * 65:(hB + 1) * 65],
                    pAB[:, 512 + off:1024], start=(kt == 0),
                    stop=(kt == nkt - 1),
                )

            def emit_attn(qb, proj_pieces):
                nkt = 4 * qb + 4
                for hp in range(2):
                    hA = 2 * hp
                    hB = 2 * hp + 1
                    uA = ps.tile([65, 512], F32, tag="u", bufs=2, name="uA")
                    uB = ps.tile([65, 512], F32, tag="u", bufs=2, name="uB")
                    prev = emit_scores(hp, qb, 0)
                    for kt in range(1, nkt):
                        cur = emit_scores(hp, qb, kt)
                        emit_exp_pv(hp, qb, prev, uA, uB, nkt)
                        if kt % 2 == 0 and proj_pieces:
                            emit_proj_piece(*proj_pieces.pop(0))
                        prev = cur
                    emit_exp_pv(hp, qb, prev, uA, uB, nkt)
                    # evacuate u to SBUF (l row rides along)
                    nc.scalar.copy(u_sb[:, hA * 4 + qb, :], uA[0:65, :])
                    nc.vector.tensor_copy(u_sb[:, hB * 4 + qb, :], uB[0:65, :])
                # flush any proj pieces the kt loop didn't have slots for
                while proj_pieces:
                    emit_proj_piece(*proj_pieces.pop(0))
                emit_chain(qb)

            # ---------------- softmax denominator chain ----------------
            def emit_chain(qb):
                nc.gpsimd.dma_start(
                    out=l_sb[32 * qb:32 * qb + 4, :],
                    in_=u_sb[64:65, :, :].rearrange(
                        "p (h q) f -> p h q f", q=4)[:, :, qb, :],
                )
                lt_ps = ps.tile([128, 16], F32, tag="gen", bufs=6, name="lt_ps")
                for sg in range(4):
                    nc.tensor.matmul(
                        lt_ps[:, sg * 4:(sg + 1) * 4],
                        l_sb[32 * qb:32 * qb + 4, sg * 128:(sg + 1) * 128],
                        id_sb[32 * qb:32 * qb + 4, 32 * qb:32 * qb + 4],
                        start=True, stop=True, is_transpose=True,
                        tile_position=(32 * qb, 0),
                    )
                nc.vector.reciprocal(linv_col[:, 16 * qb:16 * qb + 16], lt_ps[:])
                lvt_ps = ps.tile([16, 128], F32, tag="gen", bufs=6, name="lvt_ps")
                nc.tensor.matmul(
                    lvt_ps[:], linv_col[:, 16 * qb:16 * qb + 16], id_sb[:],
                    start=True, stop=True, is_transpose=True,
                )
                lr = stream.tile([16, 128], BF16, tag="lr", name="lr")
                nc.vector.tensor_copy(lr[:], lvt_ps[:])
                nc.gpsimd.dma_start(
                    out=linv_dram[16 * qb:16 * qb + 16, :], in_=lr[:]
                )
                import concourse.bass as bass_mod

                for h in range(4):
                    hp, hh = h // 2, h % 2
                    r = h * 4 + qb
                    src = bass_mod.AP(
                        linv_dram.tensor,
                        (16 * qb + h) * 128,
                        [[0, 64], [512, 4], [1, 128]],
                    )
                    lb = stream.tile([64, 512], BF16, tag="lb", bufs=3,
                                     name="lb")
                    nc.gpsimd.dma_start(out=lb[:], in_=src)
                    if hh == 0:
                        # same partition range: Pool writes u2_sb directly
                        nc.gpsimd.tensor_mul(
                            u2_sb[0:64, hp, qb * 512:(qb + 1) * 512],
                            u_sb[0:64, r, :], lb[:],
                        )
                    else:
                        # partitions 64-127: compute engines cannot cross
                        # partitions, so stage and move via DMA
                        u2t = stream.tile([64, 512], BF16, tag="u2t",
                                          bufs=2, name="u2t")
                        nc.gpsimd.tensor_mul(u2t[:], u_sb[0:64, r, :], lb[:])
                        nc.sync.dma_start(
                            out=u2_sb[64:128, hp, qb * 512:(qb + 1) * 512],
                            in_=u2t[:],
                        )

            # ---------------- output projection ----------------
            def emit_proj_piece(m, nh):
                ms = slice(m * 128, (m + 1) * 128)
                pp = ps.tile([128, 512], F32, tag="gen", bufs=6, name="pp")
                nc.tensor.matmul(
                    pp[:], u2_sb[:, 0, ms],
                    pj_sb[:, 0, nh * 512:(nh + 1) * 512],
                    start=True, stop=False,
                )
                nc.tensor.matmul(
                    pp[:], u2_sb[:, 1, ms],
                    pj_sb[:, 1, nh * 512:(nh + 1) * 512],
                    start=False, stop=True,
                )
                ob = stream.tile([128, 512], BF16, tag="ob", bufs=3, name="ob")
                if evac_eng():
                    nc.scalar.copy(ob[:], pp[:])
                else:
                    nc.vector.tensor_copy(ob[:], pp[:])
                out_queue().dma_start(
                    out=out[ms, nh * 512:(nh + 1) * 512], in_=ob[:]
                )

            def proj_pieces(qb):
                return [(m, nh) for m in range(4 * qb, 4 * qb + 4)
                        for nh in range(2)]

            qkv(0)
            qkv(1)
            emit_attn(0, [])
            qkv(2)
            emit_attn(1, proj_pieces(0))
            qkv(3)
            emit_attn(2, proj_pieces(1))
            emit_attn(3, proj_pieces(2))
            for m, nh in proj_pieces(3):
                emit_proj_piece(m, nh)
    return nc


def _get_nc(reps=1):
    key = f"nc{reps}"
    if key not in _CACHE:
        import concourse.bacc as bacc

        nc = bacc.Bacc("TRN2", target_bir_lowering=False, debug=False)
        _build_body(nc, reps=reps)
        nc.compile()
        _CACHE[key] = nc
    return _CACHE[key]


def _prep_in_maps(x, freqs_cos, freqs_sin, qkv_w, proj_w):
    import ml_dtypes

    x = np.asarray(x, dtype=np.float32)
    cos = np.asarray(freqs_cos, dtype=np.float32)
    sin = np.asarray(freqs_sin, dtype=np.float32)
    qkv_w = np.asarray(qkv_w, dtype=np.float32)
    proj_w = np.asarray(proj_w, dtype=np.float32)

    sq = np.float32((1.0 / np.sqrt(D)) ** 0.5)
    cosT = np.ascontiguousarray(cos.T) * sq  # (32, T)
    sinT = np.ascontiguousarray(sin.T) * sq
    CS = np.tile(cosT, (4, 1)).astype(np.float32)
    SN = np.tile(np.concatenate([sinT, -sinT], axis=0), (2, 1)).astype(np.float32)
    kk = np.arange(128)[:, None]
    cc = np.arange(512)[None, :]
    BM = np.full((128, 512), B16, dtype=np.float32)
    BM -= np.where((cc < 128) & (kk > cc), MBIG, 0.0).astype(np.float32)
    ident = np.eye(128, dtype=np.float32)
    perm = np.concatenate([np.arange(0, D, 2), np.arange(1, D, 2)])

    in_maps = []
    for core in range(NCORES):
        b = core // 4
        g = core % 4
        heads = [4 * g + j for j in range(HPC)]
        q_rows = np.concatenate([h * D + perm for h in heads])
        k_rows = np.concatenate([C + h * D + perm for h in heads])
        v_rows = np.concatenate([2 * C + h * D + np.arange(D) for h in heads])
        wTc = np.ascontiguousarray(
            np.concatenate(
                [qkv_w[q_rows, :], qkv_w[k_rows, :], qkv_w[v_rows, :]], axis=0
            ).T
        ).astype(ml_dtypes.bfloat16)  # (1024, 768)
        vcols = np.concatenate([h * D + np.arange(D) for h in heads])
        projTc = np.ascontiguousarray(proj_w[:, vcols].T).astype(ml_dtypes.bfloat16)
        xTc = np.ascontiguousarray(x[b].T).astype(ml_dtypes.bfloat16)
        in_maps.append(
            {
                "xT": xTc,
                "wT": wTc,
                "projT": projTc,
                "CS": CS,
                "SN": SN,
                "BM": BM,
                "ident": ident,
            }
        )
    return in_maps


def _get_runner(reps=1):
    """Build (once) a jitted 8-core shard_map executable mirroring
    bass2jax.run_bass_via_pjrt, without donation so it can be re-run for
    timing with device-resident inputs."""
    rkey = f"runner{reps}"
    if rkey in _CACHE:
        return _CACHE[rkey]
    import jax
    import concourse.mybir as mybir
    from concourse import bass2jax
    from jax.experimental.shard_map import shard_map
    from jax.sharding import Mesh, PartitionSpec

    nc = _get_nc(reps)
    bass2jax.install_neuronx_cc_hook()

    in_names = []
    out_names = []
    out_avals = []
    zero_outs = []
    pname = nc.partition_id_tensor.name if nc.partition_id_tensor else None
    for alloc in nc.m.functions[0].allocations:
        if not isinstance(alloc, mybir.MemoryLocationSet):
            continue
        name = alloc.memorylocations[0].name
        if alloc.kind == "ExternalInput":
            if name != pname:
                in_names.append(name)
        elif alloc.kind == "ExternalOutput":
            shape = tuple(alloc.tensor_shape)
            dtype = mybir.dt.np(alloc.dtype)
            out_names.append(name)
            out_avals.append(jax.core.ShapedArray(shape, dtype))
            zero_outs.append(np.zeros(shape, dtype))
    n_params = len(in_names)
    all_names = list(in_names) + list(out_names)
    if pname is not None:
        all_names.append(pname)

    def _body(*args):
        operands = list(args)
        if pname is not None:
            operands.append(bass2jax.partition_id_tensor())
        outs = bass2jax._bass_exec_p.bind(
            *operands,
            out_avals=tuple(out_avals),
            in_names=tuple(all_names),
            out_names=tuple(out_names),
            lowering_input_output_aliases=(),
            sim_require_finite=True,
            sim_require_nnan=True,
            nc=nc,
        )
        return tuple(outs)

    devices = jax.devices()[:NCORES]
    mesh = Mesh(np.asarray(devices), ("core",))
    nin = n_params + len(out_names)
    sharded_body = shard_map(
        _body,
        mesh=mesh,
        in_specs=(PartitionSpec("core"),) * nin,
        out_specs=(PartitionSpec("core"),) * len(out_names),
        check_rep=False,
    )
    sharded = jax.jit(sharded_body, keep_unused=True)
    _CACHE[rkey] = (sharded, in_names, out_names, zero_outs, mesh)
    return _CACHE[rkey]


def kernel(x, freqs_cos, freqs_sin, qkv_w, proj_w):
    import jax
    from jax.sharding import NamedSharding, PartitionSpec

    global LAST_EXEC_NS
    sharded, in_names, out_names, zero_outs, mesh = _get_runner()
    in_maps = _prep_in_maps(x, freqs_cos, freqs_sin, qkv_w, proj_w)

    concat_in = [
        np.concatenate([in_maps[c][n] for c in range(NCORES)], axis=0)
        for n in in_names
    ]
    concat_zero = [
        np.zeros((NCORES * z.shape[0], *z.shape[1:]), z.dtype) for z in zero_outs
    ]
    sharding = NamedSharding(mesh, PartitionSpec("core"))
    dev_args = [jax.device_put(a, sharding) for a in concat_in + concat_zero]

    out_arrs = sharded(*dev_args)
    jax.block_until_ready(out_arrs)

    iters = int(os.environ.get("KERNEL_TIME_ITERS", "0"))
    if iters > 0:
        import time

        sharded8 = _get_runner(reps=8)[0]
        jax.block_until_ready(sharded8(*dev_args))

        def one_round(fn):
            t0 = time.monotonic()
            for _ in range(iters):
                r = fn(*dev_args)
            jax.block_until_ready(r)
            return (time.monotonic() - t0) / iters

        diffs = []
        for _ in range(6):
            t1 = one_round(sharded)
            t8 = one_round(sharded8)
            diffs.append((t8 - t1) / 7 * 1e9)
        diffs.sort()
        LAST_EXEC_NS = diffs[len(diffs) // 2]
        _CACHE["exec_ns_min"] = diffs[0]

    out = (
        np.asarray(out_arrs[out_names.index("out")])
        .astype(np.float32)
        .reshape(NCORES, T, C)
    )
    return np.stack(
        [
            out[0] + out[1] + out[2] + out[3],
            out[4] + out[5] + out[6] + out[7],
        ],
        axis=0,
    )
